# revision 9
# baseline (speedup 1.0000x reference)
"""Trainium2 Bass kernel for a continuous-time diagonal SSM layer (S5-style).

Math (per batch sequence):
  a = exp(Lambda * step)                       (P,) complex, |a| = r, arg = theta
  Bu[l] = B_bar @ u[l]                         input projection (complex)
  x[l] = a * x[l-1] + Bu[l]                    diagonal complex scan over l
  y[l] = 2*Re(C @ x[l]) + D * u[l]

Kernel strategy (8 NeuronCores, data-parallel over batch, 2 sequences/core):
  * The complex scan is decoupled into two REAL first-order scans via phase
    modulation: with z[t] = e^{-i*theta*t} x[t], the recurrence becomes
    z[t] = r * z[t-1] + e^{-i*theta*t} Bu[t]  (r real!), which maps onto the
    hardware `tensor_tensor_scan` instruction along the free dimension.
  * Sequences are processed in chunks of T=512; phasor tables cos/sin(theta*t)
    for t in [0,T) are precomputed on host in float64 (exact mod 2pi) and kept
    resident in SBUF; chunk boundaries are re-anchored so tables are
    chunk-invariant, with the carry rotated by e^{i*theta*T} between chunks.
  * TensorE does the B/C projections in float32r (TF32-class, 4x the fp32
    rate) and also the D*u feedthrough via diagonal-weight matmuls accumulated
    into the same PSUM group; VectorE does modulate/scan/demodulate with four
    of the demod multiplies offloaded to GpSimd; ScalarE stages PSUM->SBUF.

End-to-end wall time is dominated by the axon host<->device link, so the
dispatch path is built for transfer economy, not just device time:
  * u ships to the device as fp16 in its NATURAL [B, L, H] layout (half the
    bytes, no 2.4s host-side transpose); TensorE transposes each chunk on
    device via identity matmuls, upcasting fp16 -> f32 for free on the
    PSUM->SBUF copy.
  * y ships back as fp16 and is upcast on host (the correctness gate is
    rel err < 2e-2; fp16 I/O contributes ~3e-4).
  * The jitted shard_map executable is compiled ONCE and cached, replicating
    run_bass_kernel_spmd's axon path (bass2jax run_bass_via_pjrt) minus its
    per-call retrace/relower/recompile; weight-derived constant tables are
    cached on device keyed by the weight bytes.
"""

import hashlib
import numpy as np
from contextlib import ExitStack

import jax
from jax.experimental.shard_map import shard_map
from jax.sharding import Mesh, NamedSharding, PartitionSpec

import concourse.bass as bass
import concourse.tile as tile
from concourse import bacc, mybir
from concourse.masks import make_identity
from concourse import bass2jax

# problem shape (hardcoded per contract)
BATCH, L, H, P = 16, 8192, 256, 256
NCORES = 8
BPC = BATCH // NCORES          # batch per core
T = 512                        # chunk length along L
NCHUNK = L // T
NPT = P // 128                 # partition tiles over the state dim

F32 = mybir.dt.float32
F32R = mybir.dt.float32r
F16 = mybir.dt.float16


def _build_nc():
    nc = bacc.Bacc("TRN2", target_bir_lowering=False, debug=False,
                   num_devices=NCORES)

    u_nat = nc.dram_tensor("u_nat", (BPC, L, H), F16, kind="ExternalInput")
    w_in = nc.dram_tensor("w_in", (2, 2, 128, P), F32R, kind="ExternalInput")
    c_w = nc.dram_tensor("c_w", (2, NPT, 128, H), F32R, kind="ExternalInput")
    phas = nc.dram_tensor("phas", (2, NPT, 128, T), F32, kind="ExternalInput")
    consts = nc.dram_tensor("consts", (NPT, 128, 8), F32, kind="ExternalInput")
    dg = nc.dram_tensor("dg", (2, 128, H), F32R, kind="ExternalInput")
    y_out = nc.dram_tensor("y_out", (BPC, L, H), F16, kind="ExternalOutput")

    with ExitStack() as ctx:
        tc = ctx.enter_context(tile.TileContext(nc))
        const_pool = ctx.enter_context(tc.tile_pool(name="const", bufs=1))
        un_pool = ctx.enter_context(tc.tile_pool(name="un", bufs=3))
        ut_pool = ctx.enter_context(tc.tile_pool(name="ut", bufs=3))
        g_pool = ctx.enter_context(tc.tile_pool(name="g", bufs=2))
        z_pool = ctx.enter_context(tc.tile_pool(name="z", bufs=2))
        x_pool = ctx.enter_context(tc.tile_pool(name="x", bufs=2))
        tmp_pool = ctx.enter_context(tc.tile_pool(name="tmp", bufs=4))
        carry_pool = ctx.enter_context(tc.tile_pool(name="carry", bufs=2))
        yo_pool = ctx.enter_context(tc.tile_pool(name="yo", bufs=3))
        bu_ps = ctx.enter_context(tc.tile_pool(name="bu_ps", bufs=1, space="PSUM"))
        tp_ps_pool = ctx.enter_context(tc.tile_pool(name="tp_ps", bufs=2, space="PSUM"))
        y_ps_pool = ctx.enter_context(tc.tile_pool(name="y_ps", bufs=1, space="PSUM"))

        # ---- resident constants ----
        w_in_t = const_pool.tile([128, 2, 2, P], F32R)     # [h_in_half, plane, hh, p]
        nc.sync.dma_start(w_in_t[:], w_in.rearrange("pl hh h p -> h pl hh p"))
        c_w_t = const_pool.tile([128, 2, NPT, H], F32R)    # [p_in_tile, plane, pt, h]
        nc.sync.dma_start(c_w_t[:], c_w.rearrange("pl pt p h -> p pl pt h"))
        phas_t = const_pool.tile([128, 2, NPT, T], F32)   # [p, cos/sin, pt, t]
        nc.sync.dma_start(phas_t[:], phas.rearrange("c pt p t -> p c pt t"))
        consts_t = const_pool.tile([128, NPT, 8], F32)
        nc.sync.dma_start(consts_t[:], consts.rearrange("pt p c -> p pt c"))
        dg_t = const_pool.tile([128, 2, H], F32R)
        nc.sync.dma_start(dg_t[:], dg.rearrange("hh p h -> p hh h"))

        ident_t = const_pool.tile([128, 128], F16)
        make_identity(nc, ident_t[:])

        # r broadcast tiles [128, T] per ptile (scan multiplier)
        ones_t = const_pool.tile([128, T], F32)
        nc.vector.memset(ones_t[:], 1.0)
        rbc = []
        for pt in range(NPT):
            rt = const_pool.tile([128, T], F32, tag=f"rbc{pt}")
            nc.scalar.mul(rt[:], ones_t[:], consts_t[:, pt, 0:1])
            rbc.append(rt)

        COS = [phas_t[:, 0, pt, :] for pt in range(NPT)]
        SIN = [phas_t[:, 1, pt, :] for pt in range(NPT)]

        for b in range(BPC):
            # carry state (scan-domain z at chunk end), fresh per sequence
            zl_re = [carry_pool.tile([128, 1], F32, tag=f"zlre{pt}", name=f"zlre{pt}") for pt in range(NPT)]
            zl_im = [carry_pool.tile([128, 1], F32, tag=f"zlim{pt}", name=f"zlim{pt}") for pt in range(NPT)]

            for q in range(NCHUNK):
                t0 = q * T
                # ---- load u chunk in natural layout: [t(128), s, h] fp16 ----
                un = un_pool.tile([128, 4, H], F16)
                nc.sync.dma_start(
                    un[:], u_nat[b, t0:t0 + T, :].rearrange("(s t) h -> t s h", t=128))

                # ---- device transpose to u^T chunk [h(128), hh, t] f32r ----
                ut = ut_pool.tile([128, 2, T], F32R)
                for hh in range(2):
                    tp = tp_ps_pool.tile([128, 4, 128], F16, tag="tp")
                    for s in range(4):
                        nc.tensor.transpose(
                            tp[:, s, :], un[:, s, hh * 128:(hh + 1) * 128],
                            ident_t[:])
                    nc.scalar.copy(ut[:, hh, :], tp[:])

                # ---- input projection: Bu[pt][plane] in PSUM [128, T] ----
                bu = {}
                for pt in range(NPT):
                    for pl in range(2):
                        ps = bu_ps.tile([128, T], F32, tag=f"bu{pt}{pl}")
                        for hh in range(2):
                            nc.tensor.matmul(
                                ps[:],
                                w_in_t[:, pl, hh, pt * 128:(pt + 1) * 128],
                                ut[:, hh, :],
                                start=(hh == 0), stop=(hh == 1))
                        bu[(pt, pl)] = ps

                # ---- carry hop: init = e^{i theta T} * z_last  (q>0) ----
                init_re, init_im = [], []
                for pt in range(NPT):
                    ire = carry_pool.tile([128, 1], F32, tag=f"ire{pt}")
                    iim = carry_pool.tile([128, 1], F32, tag=f"iim{pt}")
                    if q == 0:
                        nc.vector.memset(ire[:], 0.0)
                        nc.vector.memset(iim[:], 0.0)
                    else:
                        cT = consts_t[:, pt, 1:2]
                        sT = consts_t[:, pt, 2:3]
                        t_im = tmp_pool.tile([128, 1], F32, tag=f"chop{pt}")
                        # ire = cT*zl_re - sT*zl_im ; iim = sT*zl_re + cT*zl_im
                        nc.vector.tensor_scalar(t_im[:], zl_im[pt][:], sT, None,
                                                mybir.AluOpType.mult)
                        nc.vector.scalar_tensor_tensor(
                            ire[:], zl_re[pt][:], cT, t_im[:],
                            op0=mybir.AluOpType.mult, op1=mybir.AluOpType.subtract)
                        t_re = tmp_pool.tile([128, 1], F32, tag=f"chop2{pt}")
                        nc.vector.tensor_scalar(t_re[:], zl_re[pt][:], sT, None,
                                                mybir.AluOpType.mult)
                        nc.vector.scalar_tensor_tensor(
                            iim[:], zl_im[pt][:], cT, t_re[:],
                            op0=mybir.AluOpType.mult, op1=mybir.AluOpType.add)
                    init_re.append(ire)
                    init_im.append(iim)

                # ---- modulate + scan + demod per ptile ----
                x_re, x_im = [], []
                for pt in range(NPT):
                    br, bi = bu[(pt, 0)], bu[(pt, 1)]
                    t1 = tmp_pool.tile([128, T], F32, tag="t1")
                    t2 = tmp_pool.tile([128, T], F32, tag="t2")
                    g_re = g_pool.tile([128, T], F32, tag=f"gre{pt}")
                    g_im = g_pool.tile([128, T], F32, tag=f"gim{pt}")
                    # g = e^{-i theta t} * Bu
                    nc.vector.tensor_mul(t1[:], COS[pt], br[:])
                    nc.vector.tensor_mul(t2[:], SIN[pt], bi[:])
                    nc.vector.tensor_add(g_re[:], t1[:], t2[:])
                    t3 = tmp_pool.tile([128, T], F32, tag="t3")
                    t4 = tmp_pool.tile([128, T], F32, tag="t4")
                    nc.vector.tensor_mul(t3[:], COS[pt], bi[:])
                    nc.vector.tensor_mul(t4[:], SIN[pt], br[:])
                    nc.vector.tensor_sub(g_im[:], t3[:], t4[:])

                    z_re = z_pool.tile([128, T], F32, tag=f"zre{pt}")
                    z_im = z_pool.tile([128, T], F32, tag=f"zim{pt}")
                    nc.vector.tensor_tensor_scan(
                        z_re[:], rbc[pt][:], g_re[:], init_re[pt][:, 0:1],
                        mybir.AluOpType.mult, mybir.AluOpType.add)
                    nc.vector.tensor_tensor_scan(
                        z_im[:], rbc[pt][:], g_im[:], init_im[pt][:, 0:1],
                        mybir.AluOpType.mult, mybir.AluOpType.add)

                    # save carry (scan-domain, pre-demod)
                    nzl_re = carry_pool.tile([128, 1], F32, tag=f"zlre{pt}")
                    nzl_im = carry_pool.tile([128, 1], F32, tag=f"zlim{pt}")
                    nc.gpsimd.tensor_copy(nzl_re[:], z_re[:, T - 1:T])
                    nc.gpsimd.tensor_copy(nzl_im[:], z_im[:, T - 1:T])
                    zl_re[pt], zl_im[pt] = nzl_re, nzl_im

                    # x = e^{+i theta t} * z
                    xr = x_pool.tile([128, T], F32R, tag=f"xre{pt}")
                    xi = x_pool.tile([128, T], F32R, tag=f"xim{pt}")
                    t5 = tmp_pool.tile([128, T], F32, tag="t5")
                    t6 = tmp_pool.tile([128, T], F32, tag="t6")
                    nc.gpsimd.tensor_mul(t5[:], COS[pt], z_re[:])
                    nc.gpsimd.tensor_mul(t6[:], SIN[pt], z_im[:])
                    nc.vector.tensor_sub(xr[:], t5[:], t6[:])
                    t7 = tmp_pool.tile([128, T], F32, tag="t7")
                    t8 = tmp_pool.tile([128, T], F32, tag="t8")
                    nc.gpsimd.tensor_mul(t7[:], SIN[pt], z_re[:])
                    nc.gpsimd.tensor_mul(t8[:], COS[pt], z_im[:])
                    nc.vector.tensor_add(xi[:], t7[:], t8[:])
                    x_re.append(xr)
                    x_im.append(xi)

                # ---- output projection: y[t, h] += 2Re(C x) ----
                y_ps = y_ps_pool.tile([128, 4, H], F32)
                for tt in range(4):
                    first = True
                    for pt in range(NPT):
                        for pl in range(2):
                            xsrc = (x_re if pl == 0 else x_im)[pt]
                            nc.tensor.matmul(
                                y_ps[:, tt, :],
                                xsrc[:, tt * 128:(tt + 1) * 128],
                                c_w_t[:, pl, pt, :],
                                start=first, stop=False)
                            first = False
                    # feedthrough D*u as diagonal-weight matmuls (u^T already resident)
                    for hh in range(2):
                        nc.tensor.matmul(
                            y_ps[:, tt, :],
                            ut[:, hh, tt * 128:(tt + 1) * 128],
                            dg_t[:, hh, :],
                            start=False, stop=(hh == 1))

                # ---- store (fp16 to halve D2H bytes) ----
                y_sb = yo_pool.tile([128, 4, H], F16)
                nc.scalar.copy(y_sb[:], y_ps[:])
                nc.sync.dma_start(
                    y_out[b, t0:t0 + T, :].rearrange("(s t) h -> t s h", t=128),
                    y_sb[:])

    nc.compile()
    return nc


# ---------------------------------------------------------------------------
# Cached dispatch: same machinery as run_bass_kernel_spmd's axon redirect
# (bass2jax.run_bass_via_pjrt), but the jitted shard_map executable is built
# once and reused, so steady-state calls skip retrace/relower/recompile.
# ---------------------------------------------------------------------------

_STATE = None
_CONST_CACHE = {"key": None, "devs": None}


def _enumerate_io(nc):
    partition_name = (nc.partition_id_tensor.name
                      if nc.partition_id_tensor is not None else None)
    in_names, out_names, out_avals, out_shapes = [], [], [], []
    for alloc in nc.m.functions[0].allocations:
        if not isinstance(alloc, mybir.MemoryLocationSet):
            continue
        name = alloc.memorylocations[0].name
        if alloc.kind == "ExternalInput":
            if name != partition_name:
                in_names.append(name)
        elif alloc.kind == "ExternalOutput":
            shape = tuple(alloc.tensor_shape)
            dtype = mybir.dt.np(alloc.dtype)
            out_names.append(name)
            out_avals.append(jax.core.ShapedArray(shape, dtype))
            out_shapes.append((shape, dtype))
    return partition_name, in_names, out_names, out_avals, out_shapes


def _build_state():
    bass2jax.install_neuronx_cc_hook()
    nc = _build_nc()
    partition_name, in_names, out_names, out_avals, out_shapes = _enumerate_io(nc)
    n_params, n_outs = len(in_names), len(out_names)
    bind_names = list(in_names)
    if partition_name is not None:
        bind_names.append(partition_name)

    devices = jax.devices()[:NCORES]
    mesh = Mesh(np.asarray(devices), ("core",))
    shard = NamedSharding(mesh, PartitionSpec("core"))

    # Outputs bind to the custom call's results via out_names (same contract
    # bass_jit uses) — no zero-initialized donation operands needed, since the
    # kernel writes every element of y_out.
    def _body(*args):
        operands = list(args)
        if partition_name is not None:
            operands.append(bass2jax.partition_id_tensor())
        outs = bass2jax._bass_exec_p.bind(
            *operands,
            out_avals=tuple(out_avals),
            in_names=tuple(bind_names),
            out_names=tuple(out_names),
            lowering_input_output_aliases=(),
            sim_require_finite=True,
            sim_require_nnan=True,
            nc=nc,
        )
        return tuple(outs)

    # global (concatenated-over-cores) avals for lowering
    in_shapes = {}
    for alloc in nc.m.functions[0].allocations:
        if isinstance(alloc, mybir.MemoryLocationSet) and alloc.kind == "ExternalInput":
            in_shapes[alloc.memorylocations[0].name] = (
                tuple(alloc.tensor_shape), mybir.dt.np(alloc.dtype))
    lower_args = []
    for name in in_names:
        shape, dtype = in_shapes[name]
        lower_args.append(jax.ShapeDtypeStruct(
            (NCORES * shape[0],) + shape[1:], dtype, sharding=shard))

    in_specs = (PartitionSpec("core"),) * n_params
    out_specs = (PartitionSpec("core"),) * n_outs

    def compile_fn():
        jitted = jax.jit(
            shard_map(_body, mesh=mesh, in_specs=in_specs,
                      out_specs=out_specs, check_rep=False),
            keep_unused=True)
        return jitted.lower(*lower_args).compile()

    try:
        compiled = bass2jax.fast_dispatch_compile(compile_fn)
    except Exception:
        compiled = compile_fn()

    from concurrent.futures import ThreadPoolExecutor
    return {
        "nc": nc, "compiled": compiled, "shard": shard,
        "in_names": in_names, "out_names": out_names,
        "u16_buf": np.empty((BATCH, L, H), np.float16),
        "executor": ThreadPoolExecutor(max_workers=8),
    }


def _get_state():
    global _STATE
    if _STATE is None:
        _STATE = _build_state()
    return _STATE


def _host_prep(Lambda_re, Lambda_im, B, C, D, log_step):
    """Precompute device constant tables in float64."""
    Lam = Lambda_re.astype(np.float64) + 1j * Lambda_im.astype(np.float64)
    step = np.exp(log_step[:, 0].astype(np.float64))
    a = np.exp(Lam * step)
    r = np.abs(a)
    theta = Lam.imag * step
    Bb = ((a - 1.0) / Lam)[:, None] * (
        B[..., 0].astype(np.float64) + 1j * B[..., 1].astype(np.float64))
    Ct = C[..., 0].astype(np.float64) + 1j * C[..., 1].astype(np.float64)

    W = np.stack([Bb.real, Bb.imag]).astype(np.float32)        # [2, P, H]
    # w_in[pl, hh, hi, p] = W[pl, p, hh*128+hi]
    w_in = np.ascontiguousarray(
        W.transpose(0, 2, 1).reshape(2, 2, 128, P)).astype(np.float32)
    # c_w[pl, pt, pi, h]: pl=0 -> 2*C_re[h, p], pl=1 -> -2*C_im[h, p]
    C2 = np.stack([2.0 * Ct.real, -2.0 * Ct.imag])              # [2, H, P]
    c_w = np.ascontiguousarray(
        C2.transpose(0, 2, 1).reshape(2, NPT, 128, H)).astype(np.float32)

    t = np.arange(T, dtype=np.float64)
    ang = np.mod(np.outer(theta, t), 2 * np.pi)                 # [P, T]
    phas = np.stack([np.cos(ang), np.sin(ang)]).reshape(2, NPT, 128, T)
    phas = np.ascontiguousarray(phas).astype(np.float32)

    angT = np.mod(theta * T, 2 * np.pi)
    consts = np.zeros((NPT, 128, 8), np.float64)
    consts[:, :, 0] = r.reshape(NPT, 128)
    consts[:, :, 1] = np.cos(angT).reshape(NPT, 128)
    consts[:, :, 2] = np.sin(angT).reshape(NPT, 128)
    consts = consts.astype(np.float32)

    dgm = np.zeros((2, 128, H), np.float32)
    for hh in range(2):
        for hi in range(128):
            dgm[hh, hi, hh * 128 + hi] = D[hh * 128 + hi]
    return {"w_in": w_in, "c_w": c_w, "phas": phas,
            "consts": consts, "dg": dgm}


def _get_const_devs(state, Lambda_re, Lambda_im, B, C, D, log_step):
    """Device-resident constant tables, cached keyed on the weight bytes."""
    h = hashlib.sha1()
    for a in (Lambda_re, Lambda_im, B, C, D, log_step):
        h.update(np.ascontiguousarray(a).tobytes())
    key = h.hexdigest()
    if _CONST_CACHE["key"] == key:
        return _CONST_CACHE["devs"]
    tables = _host_prep(Lambda_re, Lambda_im, B, C, D, log_step)
    devs = {}
    for name, arr in tables.items():
        rep = np.broadcast_to(
            arr[None], (NCORES,) + arr.shape).reshape(
                (NCORES * arr.shape[0],) + arr.shape[1:])
        devs[name] = jax.device_put(np.ascontiguousarray(rep), state["shard"])
    for v in devs.values():
        v.block_until_ready()
    _CONST_CACHE["key"] = key
    _CONST_CACHE["devs"] = devs
    return devs


def _fetch_upcast(state, y_dev):
    """Fetch the sharded fp16 result, upcasting shards to f32 while later
    shards are still in flight on the relay."""
    y32 = np.empty((BATCH, L, H), np.float32)
    ex = state["executor"]
    shards = sorted(y_dev.addressable_shards, key=lambda s: s.index[0].start or 0)
    futs = [(s.index[0].start or 0, ex.submit(np.asarray, s.data)) for s in shards]
    for start, f in futs:
        part = f.result()
        y32[start:start + part.shape[0]] = part
    return y32


def kernel(input_sequence, Lambda_re, Lambda_im, B, C, D, log_step):
    state = _get_state()
    const_devs = _get_const_devs(
        state, np.asarray(Lambda_re), np.asarray(Lambda_im), np.asarray(B),
        np.asarray(C), np.asarray(D), np.asarray(log_step))

    # parallel f32 -> fp16 cast into a reusable staging buffer
    src = np.asarray(input_sequence)
    u16 = state["u16_buf"]
    ex = state["executor"]
    list(ex.map(
        lambda c: np.copyto(u16[BPC * c:BPC * (c + 1)],
                            src[BPC * c:BPC * (c + 1)], casting="unsafe"),
        range(NCORES)))
    u_dev = jax.device_put(u16, state["shard"])

    args = [u_dev if name == "u_nat" else const_devs[name]
            for name in state["in_names"]]
    outs = state["compiled"](*args)
    return _fetch_upcast(state, outs[0])


if __name__ == "__main__":
    rng = np.random.default_rng(0)
    print("smoke test: building kernel...")
    _get_state()
    print("built ok")


# revision 11
# speedup vs baseline: 1.0601x; 1.0601x over previous
"""Trainium2 Bass kernel for a continuous-time diagonal SSM layer (S5-style).

Math (per batch sequence):
  a = exp(Lambda * step)                       (P,) complex, |a| = r, arg = theta
  Bu[l] = B_bar @ u[l]                         input projection (complex)
  x[l] = a * x[l-1] + Bu[l]                    diagonal complex scan over l
  y[l] = 2*Re(C @ x[l]) + D * u[l]

Kernel strategy (8 NeuronCores, data-parallel over batch, 2 sequences/core):
  * The complex scan is decoupled into two REAL first-order scans via phase
    modulation: with z[t] = e^{-i*theta*t} x[t], the recurrence becomes
    z[t] = r * z[t-1] + e^{-i*theta*t} Bu[t]  (r real!), which maps onto the
    hardware `tensor_tensor_scan` instruction along the free dimension.
  * Sequences are processed in chunks of T=512; phasor tables cos/sin(theta*t)
    for t in [0,T) are precomputed on host in float64 (exact mod 2pi) and kept
    resident in SBUF; chunk boundaries are re-anchored so tables are
    chunk-invariant, with the carry rotated by e^{i*theta*T} between chunks.
  * TensorE does the B/C projections in float32r (TF32-class, 4x the fp32
    rate) and also the D*u feedthrough via diagonal-weight matmuls accumulated
    into the same PSUM group; VectorE does modulate/scan/demodulate with four
    of the demod multiplies offloaded to GpSimd; ScalarE stages PSUM->SBUF.

End-to-end wall time is dominated by the axon host<->device link, so the
dispatch path is built for transfer economy, not just device time:
  * u ships to the device as fp16 in its NATURAL [B, L, H] layout (half the
    bytes, no 2.4s host-side transpose); TensorE transposes each chunk on
    device via identity matmuls, upcasting fp16 -> f32 for free on the
    PSUM->SBUF copy.
  * y ships back as fp16 and is upcast on host (the correctness gate is
    rel err < 2e-2; fp16 I/O contributes ~3e-4).
  * The jitted shard_map executable is compiled ONCE and cached, replicating
    run_bass_kernel_spmd's axon path (bass2jax run_bass_via_pjrt) minus its
    per-call retrace/relower/recompile; weight-derived constant tables are
    cached on device keyed by the weight bytes.
"""

import hashlib
import time
import numpy as np
from contextlib import ExitStack

import jax
from jax.experimental.shard_map import shard_map
from jax.sharding import Mesh, NamedSharding, PartitionSpec

import concourse.bass as bass
import concourse.tile as tile
from concourse import bacc, mybir
from concourse.masks import make_identity
from concourse import bass2jax

# problem shape (hardcoded per contract)
BATCH, L, H, P = 16, 8192, 256, 256
NCORES = 8
BPC = BATCH // NCORES          # batch per core
T = 512                        # chunk length along L
NCHUNK = L // T
NPT = P // 128                 # partition tiles over the state dim

F32 = mybir.dt.float32
F32R = mybir.dt.float32r
F16 = mybir.dt.float16


def _build_nc():
    nc = bacc.Bacc("TRN2", target_bir_lowering=False, debug=False,
                   num_devices=NCORES)

    u_nat = nc.dram_tensor("u_nat", (BPC, L, H), F16, kind="ExternalInput")
    w_in = nc.dram_tensor("w_in", (2, 2, 128, P), F32R, kind="ExternalInput")
    c_w = nc.dram_tensor("c_w", (2, NPT, 128, H), F32R, kind="ExternalInput")
    phas = nc.dram_tensor("phas", (2, NPT, 128, T), F32, kind="ExternalInput")
    consts = nc.dram_tensor("consts", (NPT, 128, 8), F32, kind="ExternalInput")
    dg = nc.dram_tensor("dg", (2, 128, H), F32R, kind="ExternalInput")
    y_out = nc.dram_tensor("y_out", (BPC, L, H), F16, kind="ExternalOutput")

    with ExitStack() as ctx:
        tc = ctx.enter_context(tile.TileContext(nc))
        const_pool = ctx.enter_context(tc.tile_pool(name="const", bufs=1))
        un_pool = ctx.enter_context(tc.tile_pool(name="un", bufs=3))
        ut_pool = ctx.enter_context(tc.tile_pool(name="ut", bufs=3))
        g_pool = ctx.enter_context(tc.tile_pool(name="g", bufs=2))
        z_pool = ctx.enter_context(tc.tile_pool(name="z", bufs=2))
        x_pool = ctx.enter_context(tc.tile_pool(name="x", bufs=2))
        tmp_pool = ctx.enter_context(tc.tile_pool(name="tmp", bufs=4))
        carry_pool = ctx.enter_context(tc.tile_pool(name="carry", bufs=2))
        yo_pool = ctx.enter_context(tc.tile_pool(name="yo", bufs=3))
        bu_ps = ctx.enter_context(tc.tile_pool(name="bu_ps", bufs=1, space="PSUM"))
        tp_ps_pool = ctx.enter_context(tc.tile_pool(name="tp_ps", bufs=2, space="PSUM"))
        y_ps_pool = ctx.enter_context(tc.tile_pool(name="y_ps", bufs=1, space="PSUM"))

        # ---- resident constants ----
        w_in_t = const_pool.tile([128, 2, 2, P], F32R)     # [h_in_half, plane, hh, p]
        nc.sync.dma_start(w_in_t[:], w_in.rearrange("pl hh h p -> h pl hh p"))
        c_w_t = const_pool.tile([128, 2, NPT, H], F32R)    # [p_in_tile, plane, pt, h]
        nc.sync.dma_start(c_w_t[:], c_w.rearrange("pl pt p h -> p pl pt h"))
        phas_t = const_pool.tile([128, 2, NPT, T], F32)   # [p, cos/sin, pt, t]
        nc.sync.dma_start(phas_t[:], phas.rearrange("c pt p t -> p c pt t"))
        consts_t = const_pool.tile([128, NPT, 8], F32)
        nc.sync.dma_start(consts_t[:], consts.rearrange("pt p c -> p pt c"))
        dg_t = const_pool.tile([128, 2, H], F32R)
        nc.sync.dma_start(dg_t[:], dg.rearrange("hh p h -> p hh h"))

        ident_t = const_pool.tile([128, 128], F16)
        make_identity(nc, ident_t[:])

        # r broadcast tiles [128, T] per ptile (scan multiplier)
        ones_t = const_pool.tile([128, T], F32)
        nc.vector.memset(ones_t[:], 1.0)
        rbc = []
        for pt in range(NPT):
            rt = const_pool.tile([128, T], F32, tag=f"rbc{pt}")
            nc.scalar.mul(rt[:], ones_t[:], consts_t[:, pt, 0:1])
            rbc.append(rt)

        COS = [phas_t[:, 0, pt, :] for pt in range(NPT)]
        SIN = [phas_t[:, 1, pt, :] for pt in range(NPT)]

        for b in range(BPC):
            # carry state (scan-domain z at chunk end), fresh per sequence
            zl_re = [carry_pool.tile([128, 1], F32, tag=f"zlre{pt}", name=f"zlre{pt}") for pt in range(NPT)]
            zl_im = [carry_pool.tile([128, 1], F32, tag=f"zlim{pt}", name=f"zlim{pt}") for pt in range(NPT)]

            for q in range(NCHUNK):
                t0 = q * T
                # ---- load u chunk in natural layout: [t(128), s, h] fp16 ----
                un = un_pool.tile([128, 4, H], F16)
                nc.sync.dma_start(
                    un[:], u_nat[b, t0:t0 + T, :].rearrange("(s t) h -> t s h", t=128))

                # ---- device transpose to u^T chunk [h(128), hh, t] f32r ----
                ut = ut_pool.tile([128, 2, T], F32R)
                for hh in range(2):
                    tp = tp_ps_pool.tile([128, 4, 128], F16, tag="tp")
                    for s in range(4):
                        nc.tensor.transpose(
                            tp[:, s, :], un[:, s, hh * 128:(hh + 1) * 128],
                            ident_t[:])
                    nc.scalar.copy(ut[:, hh, :], tp[:])

                # ---- input projection: Bu[pt][plane] in PSUM [128, T] ----
                bu = {}
                for pt in range(NPT):
                    for pl in range(2):
                        ps = bu_ps.tile([128, T], F32, tag=f"bu{pt}{pl}")
                        for hh in range(2):
                            nc.tensor.matmul(
                                ps[:],
                                w_in_t[:, pl, hh, pt * 128:(pt + 1) * 128],
                                ut[:, hh, :],
                                start=(hh == 0), stop=(hh == 1))
                        bu[(pt, pl)] = ps

                # ---- carry hop: init = e^{i theta T} * z_last  (q>0) ----
                init_re, init_im = [], []
                for pt in range(NPT):
                    ire = carry_pool.tile([128, 1], F32, tag=f"ire{pt}")
                    iim = carry_pool.tile([128, 1], F32, tag=f"iim{pt}")
                    if q == 0:
                        nc.vector.memset(ire[:], 0.0)
                        nc.vector.memset(iim[:], 0.0)
                    else:
                        cT = consts_t[:, pt, 1:2]
                        sT = consts_t[:, pt, 2:3]
                        t_im = tmp_pool.tile([128, 1], F32, tag=f"chop{pt}")
                        # ire = cT*zl_re - sT*zl_im ; iim = sT*zl_re + cT*zl_im
                        nc.vector.tensor_scalar(t_im[:], zl_im[pt][:], sT, None,
                                                mybir.AluOpType.mult)
                        nc.vector.scalar_tensor_tensor(
                            ire[:], zl_re[pt][:], cT, t_im[:],
                            op0=mybir.AluOpType.mult, op1=mybir.AluOpType.subtract)
                        t_re = tmp_pool.tile([128, 1], F32, tag=f"chop2{pt}")
                        nc.vector.tensor_scalar(t_re[:], zl_re[pt][:], sT, None,
                                                mybir.AluOpType.mult)
                        nc.vector.scalar_tensor_tensor(
                            iim[:], zl_im[pt][:], cT, t_re[:],
                            op0=mybir.AluOpType.mult, op1=mybir.AluOpType.add)
                    init_re.append(ire)
                    init_im.append(iim)

                # ---- modulate + scan + demod per ptile ----
                x_re, x_im = [], []
                for pt in range(NPT):
                    br, bi = bu[(pt, 0)], bu[(pt, 1)]
                    t1 = tmp_pool.tile([128, T], F32, tag="t1")
                    t2 = tmp_pool.tile([128, T], F32, tag="t2")
                    g_re = g_pool.tile([128, T], F32, tag=f"gre{pt}")
                    g_im = g_pool.tile([128, T], F32, tag=f"gim{pt}")
                    # g = e^{-i theta t} * Bu
                    nc.vector.tensor_mul(t1[:], COS[pt], br[:])
                    nc.vector.tensor_mul(t2[:], SIN[pt], bi[:])
                    nc.vector.tensor_add(g_re[:], t1[:], t2[:])
                    t3 = tmp_pool.tile([128, T], F32, tag="t3")
                    t4 = tmp_pool.tile([128, T], F32, tag="t4")
                    nc.vector.tensor_mul(t3[:], COS[pt], bi[:])
                    nc.vector.tensor_mul(t4[:], SIN[pt], br[:])
                    nc.vector.tensor_sub(g_im[:], t3[:], t4[:])

                    z_re = z_pool.tile([128, T], F32, tag=f"zre{pt}")
                    z_im = z_pool.tile([128, T], F32, tag=f"zim{pt}")
                    nc.vector.tensor_tensor_scan(
                        z_re[:], rbc[pt][:], g_re[:], init_re[pt][:, 0:1],
                        mybir.AluOpType.mult, mybir.AluOpType.add)
                    nc.vector.tensor_tensor_scan(
                        z_im[:], rbc[pt][:], g_im[:], init_im[pt][:, 0:1],
                        mybir.AluOpType.mult, mybir.AluOpType.add)

                    # save carry (scan-domain, pre-demod)
                    nzl_re = carry_pool.tile([128, 1], F32, tag=f"zlre{pt}")
                    nzl_im = carry_pool.tile([128, 1], F32, tag=f"zlim{pt}")
                    nc.gpsimd.tensor_copy(nzl_re[:], z_re[:, T - 1:T])
                    nc.gpsimd.tensor_copy(nzl_im[:], z_im[:, T - 1:T])
                    zl_re[pt], zl_im[pt] = nzl_re, nzl_im

                    # x = e^{+i theta t} * z
                    xr = x_pool.tile([128, T], F32R, tag=f"xre{pt}")
                    xi = x_pool.tile([128, T], F32R, tag=f"xim{pt}")
                    t5 = tmp_pool.tile([128, T], F32, tag="t5")
                    t6 = tmp_pool.tile([128, T], F32, tag="t6")
                    nc.gpsimd.tensor_mul(t5[:], COS[pt], z_re[:])
                    nc.gpsimd.tensor_mul(t6[:], SIN[pt], z_im[:])
                    nc.vector.tensor_sub(xr[:], t5[:], t6[:])
                    t7 = tmp_pool.tile([128, T], F32, tag="t7")
                    t8 = tmp_pool.tile([128, T], F32, tag="t8")
                    nc.gpsimd.tensor_mul(t7[:], SIN[pt], z_re[:])
                    nc.gpsimd.tensor_mul(t8[:], COS[pt], z_im[:])
                    nc.vector.tensor_add(xi[:], t7[:], t8[:])
                    x_re.append(xr)
                    x_im.append(xi)

                # ---- output projection: y[t, h] += 2Re(C x) ----
                y_ps = y_ps_pool.tile([128, 4, H], F32)
                for tt in range(4):
                    first = True
                    for pt in range(NPT):
                        for pl in range(2):
                            xsrc = (x_re if pl == 0 else x_im)[pt]
                            nc.tensor.matmul(
                                y_ps[:, tt, :],
                                xsrc[:, tt * 128:(tt + 1) * 128],
                                c_w_t[:, pl, pt, :],
                                start=first, stop=False)
                            first = False
                    # feedthrough D*u as diagonal-weight matmuls (u^T already resident)
                    for hh in range(2):
                        nc.tensor.matmul(
                            y_ps[:, tt, :],
                            ut[:, hh, tt * 128:(tt + 1) * 128],
                            dg_t[:, hh, :],
                            start=False, stop=(hh == 1))

                # ---- store (fp16 to halve D2H bytes) ----
                y_sb = yo_pool.tile([128, 4, H], F16)
                nc.scalar.copy(y_sb[:], y_ps[:])
                nc.sync.dma_start(
                    y_out[b, t0:t0 + T, :].rearrange("(s t) h -> t s h", t=128),
                    y_sb[:])

    nc.compile()
    return nc


# ---------------------------------------------------------------------------
# Cached dispatch: same machinery as run_bass_kernel_spmd's axon redirect
# (bass2jax.run_bass_via_pjrt), but the jitted shard_map executable is built
# once and reused, so steady-state calls skip retrace/relower/recompile.
# ---------------------------------------------------------------------------

_STATE = None
_CONST_CACHE = {"key": None, "devs": None}


def _enumerate_io(nc):
    partition_name = (nc.partition_id_tensor.name
                      if nc.partition_id_tensor is not None else None)
    in_names, out_names, out_avals, out_shapes = [], [], [], []
    for alloc in nc.m.functions[0].allocations:
        if not isinstance(alloc, mybir.MemoryLocationSet):
            continue
        name = alloc.memorylocations[0].name
        if alloc.kind == "ExternalInput":
            if name != partition_name:
                in_names.append(name)
        elif alloc.kind == "ExternalOutput":
            shape = tuple(alloc.tensor_shape)
            dtype = mybir.dt.np(alloc.dtype)
            out_names.append(name)
            out_avals.append(jax.core.ShapedArray(shape, dtype))
            out_shapes.append((shape, dtype))
    return partition_name, in_names, out_names, out_avals, out_shapes


def _build_state():
    bass2jax.install_neuronx_cc_hook()
    nc = _build_nc()
    partition_name, in_names, out_names, out_avals, out_shapes = _enumerate_io(nc)
    n_params, n_outs = len(in_names), len(out_names)
    bind_names = list(in_names)
    if partition_name is not None:
        bind_names.append(partition_name)

    devices = jax.devices()[:NCORES]
    mesh = Mesh(np.asarray(devices), ("core",))
    shard = NamedSharding(mesh, PartitionSpec("core"))

    # Outputs bind to the custom call's results via out_names (same contract
    # bass_jit uses) — no zero-initialized donation operands needed, since the
    # kernel writes every element of y_out.
    def _body(*args):
        operands = list(args)
        if partition_name is not None:
            operands.append(bass2jax.partition_id_tensor())
        outs = bass2jax._bass_exec_p.bind(
            *operands,
            out_avals=tuple(out_avals),
            in_names=tuple(bind_names),
            out_names=tuple(out_names),
            lowering_input_output_aliases=(),
            sim_require_finite=True,
            sim_require_nnan=True,
            nc=nc,
        )
        return tuple(outs)

    # global (concatenated-over-cores) avals for lowering
    in_shapes = {}
    for alloc in nc.m.functions[0].allocations:
        if isinstance(alloc, mybir.MemoryLocationSet) and alloc.kind == "ExternalInput":
            in_shapes[alloc.memorylocations[0].name] = (
                tuple(alloc.tensor_shape), mybir.dt.np(alloc.dtype))
    lower_args = []
    for name in in_names:
        shape, dtype = in_shapes[name]
        lower_args.append(jax.ShapeDtypeStruct(
            (NCORES * shape[0],) + shape[1:], dtype, sharding=shard))

    in_specs = (PartitionSpec("core"),) * n_params
    out_specs = (PartitionSpec("core"),) * n_outs

    def compile_fn():
        jitted = jax.jit(
            shard_map(_body, mesh=mesh, in_specs=in_specs,
                      out_specs=out_specs, check_rep=False),
            keep_unused=True)
        return jitted.lower(*lower_args).compile()

    try:
        compiled = bass2jax.fast_dispatch_compile(compile_fn)
    except Exception:
        compiled = compile_fn()

    from concurrent.futures import ThreadPoolExecutor
    return {
        "nc": nc, "compiled": compiled, "shard": shard,
        "in_names": in_names, "out_names": out_names,
        "u16_buf": np.empty((BATCH, L, H), np.float16),
        "executor": ThreadPoolExecutor(max_workers=8),
    }


def _get_state():
    global _STATE
    if _STATE is None:
        _STATE = _build_state()
    return _STATE


def _host_prep(Lambda_re, Lambda_im, B, C, D, log_step):
    """Precompute device constant tables in float64."""
    Lam = Lambda_re.astype(np.float64) + 1j * Lambda_im.astype(np.float64)
    step = np.exp(log_step[:, 0].astype(np.float64))
    a = np.exp(Lam * step)
    r = np.abs(a)
    theta = Lam.imag * step
    Bb = ((a - 1.0) / Lam)[:, None] * (
        B[..., 0].astype(np.float64) + 1j * B[..., 1].astype(np.float64))
    Ct = C[..., 0].astype(np.float64) + 1j * C[..., 1].astype(np.float64)

    W = np.stack([Bb.real, Bb.imag]).astype(np.float32)        # [2, P, H]
    # w_in[pl, hh, hi, p] = W[pl, p, hh*128+hi]
    w_in = np.ascontiguousarray(
        W.transpose(0, 2, 1).reshape(2, 2, 128, P)).astype(np.float32)
    # c_w[pl, pt, pi, h]: pl=0 -> 2*C_re[h, p], pl=1 -> -2*C_im[h, p]
    C2 = np.stack([2.0 * Ct.real, -2.0 * Ct.imag])              # [2, H, P]
    c_w = np.ascontiguousarray(
        C2.transpose(0, 2, 1).reshape(2, NPT, 128, H)).astype(np.float32)

    t = np.arange(T, dtype=np.float64)
    ang = np.mod(np.outer(theta, t), 2 * np.pi)                 # [P, T]
    phas = np.stack([np.cos(ang), np.sin(ang)]).reshape(2, NPT, 128, T)
    phas = np.ascontiguousarray(phas).astype(np.float32)

    angT = np.mod(theta * T, 2 * np.pi)
    consts = np.zeros((NPT, 128, 8), np.float64)
    consts[:, :, 0] = r.reshape(NPT, 128)
    consts[:, :, 1] = np.cos(angT).reshape(NPT, 128)
    consts[:, :, 2] = np.sin(angT).reshape(NPT, 128)
    consts = consts.astype(np.float32)

    dgm = np.zeros((2, 128, H), np.float32)
    for hh in range(2):
        for hi in range(128):
            dgm[hh, hi, hh * 128 + hi] = D[hh * 128 + hi]
    return {"w_in": w_in, "c_w": c_w, "phas": phas,
            "consts": consts, "dg": dgm}


def _get_const_devs(state, Lambda_re, Lambda_im, B, C, D, log_step):
    """Device-resident constant tables, cached keyed on the weight bytes."""
    h = hashlib.sha1()
    for a in (Lambda_re, Lambda_im, B, C, D, log_step):
        h.update(np.ascontiguousarray(a).tobytes())
    key = h.hexdigest()
    if _CONST_CACHE["key"] == key:
        return _CONST_CACHE["devs"]
    tables = _host_prep(Lambda_re, Lambda_im, B, C, D, log_step)
    devs = {}
    for name, arr in tables.items():
        rep = np.broadcast_to(
            arr[None], (NCORES,) + arr.shape).reshape(
                (NCORES * arr.shape[0],) + arr.shape[1:])
        devs[name] = jax.device_put(np.ascontiguousarray(rep), state["shard"])
    for v in devs.values():
        v.block_until_ready()
    _CONST_CACHE["key"] = key
    _CONST_CACHE["devs"] = devs
    return devs


def _fetch_upcast(state, y_dev):
    """Fetch the sharded fp16 result, upcasting shards to f32 while later
    shards are still in flight on the relay."""
    y32 = np.empty((BATCH, L, H), np.float32)
    ex = state["executor"]
    shards = sorted(y_dev.addressable_shards, key=lambda s: s.index[0].start or 0)
    futs = [(s.index[0].start or 0, ex.submit(np.asarray, s.data)) for s in shards]
    for start, f in futs:
        part = f.result()
        y32[start:start + part.shape[0]] = part
    return y32


def kernel(input_sequence, Lambda_re, Lambda_im, B, C, D, log_step):
    """Full-input entry point; retries to ride out transient relay failures.

    Attempt 1 failure: drop cached device-side constant tables (they may have
    been lost with the terminal) and retry. Attempt 2 failure: rebuild the
    whole state, including the compiled executable, and retry once more.
    """
    global _STATE
    last_err = None
    for attempt in range(3):
        try:
            return _kernel_once(
                input_sequence, Lambda_re, Lambda_im, B, C, D, log_step)
        except Exception as e:  # transient relay/terminal errors
            last_err = e
            _CONST_CACHE["key"] = None
            _CONST_CACHE["devs"] = None
            if attempt >= 1:
                _STATE = None
            time.sleep(2.0)
    raise last_err


def _kernel_once(input_sequence, Lambda_re, Lambda_im, B, C, D, log_step):
    state = _get_state()
    const_devs = _get_const_devs(
        state, np.asarray(Lambda_re), np.asarray(Lambda_im), np.asarray(B),
        np.asarray(C), np.asarray(D), np.asarray(log_step))

    # parallel f32 -> fp16 cast into a reusable staging buffer
    src = np.asarray(input_sequence)
    u16 = state["u16_buf"]
    ex = state["executor"]
    list(ex.map(
        lambda c: np.copyto(u16[BPC * c:BPC * (c + 1)],
                            src[BPC * c:BPC * (c + 1)], casting="unsafe"),
        range(NCORES)))
    u_dev = jax.device_put(u16, state["shard"])

    args = [u_dev if name == "u_nat" else const_devs[name]
            for name in state["in_names"]]
    outs = state["compiled"](*args)
    return _fetch_upcast(state, outs[0])


if __name__ == "__main__":
    rng = np.random.default_rng(0)
    print("smoke test: building kernel...")
    _get_state()
    print("built ok")
